# revision 14
# baseline (speedup 1.0000x reference)
"""Trainium2 Bass kernel for GAT(128->32x8, concat) + SAGEConv(256->64, mean).

Self-contained: host-side index preprocessing + SPMD Bass program on 8 cores.

Sharding: dst-node graph partition. Core c owns padded node rows
[PC*c, PC*(c+1)), PC = NB*128. Edges are sorted by dst on the host; each core
processes the ~E/8 edges whose dst it owns, gathering source-node features by
edge via dma_gather from HBM tables (layer 1: h|a_s table built on device in
phase 0; layer 2: x32 table assembled via AllGather). Segment softmax-sums are
one-hot matmuls on the TensorEngine with fused denominator columns; the
softmax max-shift is dropped (shift-invariant, logits are O(1)).

SPMD uniformity: one program runs on all 8 cores; all per-core variation
(gather indices, one-hot masks, degree reciprocals) is carried in input
tensors. Tile counts per (block, half) are the max over cores; phantom slots
gather row 0 and carry all-zero mask columns.
"""
import os
import numpy as np
from contextlib import ExitStack

# ---------------------------------------------------------------- config ----
N = 50000
E = 800000
IN = 128
HID = 32
HEADS = 8
CAT = HEADS * HID   # 256
OUT = 64
NCORES = 8
BLK = 128
NB = 49             # dst blocks per core
PC = NB * BLK       # 6272 nodes per core
NPAD = PC * NCORES  # 50176 padded nodes
NCHUNK = NPAD // BLK  # 392 phase-0 chunks
SPLIT = 32768       # int16 dma_gather index limit
G = 2               # dst blocks per gather-call group
TROW = 384          # table1 row: 256 f16 h + 8 f32 a_s (16 f16 slots) + pad -> 768B


# ------------------------------------------------------------- host prep ----
def _wrap_idx16(idx):
    """dma_gather index layout: element i -> partition i%16, col i//16,
    replicated across the 8 groups of 16 partitions. idx length % 16 == 0."""
    n = len(idx)
    out = np.asarray(idx, np.int64).reshape(n // 16, 16).T.astype(np.int16)
    return np.tile(out, (8, 1))  # [128, n//16]


def prepare(edge_index):
    """Index-only preprocessing: edge sort, A/B split, uniform tile plan,
    one-hot masks, gather index arrays, degree reciprocals."""
    src = np.asarray(edge_index[0], np.int64)
    dst = np.asarray(edge_index[1], np.int64)

    deg = np.bincount(dst, minlength=NPAD).astype(np.float32)
    deg_inv = (1.0 / np.maximum(deg, 1.0)).reshape(NCORES, PC)

    order = np.argsort(dst, kind="stable")
    s_sorted, d_sorted = src[order], dst[order]

    # per (core, block, half) edge lists: (src_idx_for_gather, dst_local)
    lists = [[[None, None] for _ in range(NB)] for _ in range(NCORES)]
    for c in range(NCORES):
        lo, hi = np.searchsorted(d_sorted, [c * PC, (c + 1) * PC])
        sc, dc = s_sorted[lo:hi], d_sorted[lo:hi]
        blk = (dc - c * PC) // BLK
        bounds = np.searchsorted(blk, np.arange(NB + 1))
        for b in range(NB):
            sb = sc[bounds[b]:bounds[b + 1]]
            db = dc[bounds[b]:bounds[b + 1]] - c * PC - b * BLK
            a = sb < SPLIT
            lists[c][b][0] = (sb[a], db[a])
            lists[c][b][1] = (sb[~a] - SPLIT, db[~a])

    # uniform tile counts
    tiles = np.zeros((NB, 2), np.int64)
    for b in range(NB):
        for h in range(2):
            mx = max(len(lists[c][b][h][0]) for c in range(NCORES))
            tiles[b, h] = max(-(-mx // BLK), 1)

    grp_blocks = [list(range(g0, min(g0 + G, NB))) for g0 in range(0, NB, G)]
    ngrp = len(grp_blocks)
    # gather call sizes per (group, half)
    call_n = np.array([[int(tiles[bs, h].sum()) * BLK for h in range(2)]
                       for bs in [np.array(b) for b in grp_blocks]], np.int64)

    # tile meta in emission order: per group, per block, A tiles then B tiles
    # entry: (group, half, block, slot_in_call, first_of_block, last_of_block)
    meta = []
    for g, bs in enumerate(grp_blocks):
        slot = [0, 0]
        for b in bs:
            tot = tiles[b, 0] + tiles[b, 1]
            k = 0
            for h in range(2):
                for _ in range(tiles[b, h]):
                    meta.append((g, h, b, slot[h], k == 0, k == tot - 1))
                    slot[h] += 1
                    k += 1
    total_tiles = len(meta)

    per_core = []
    for c in range(NCORES):
        idx_parts = []     # wrapped idx arrays, call order (g-major, A then B)
        S = np.zeros((total_tiles, BLK, BLK), np.float16)
        ST = np.zeros((total_tiles, BLK, BLK), np.float16)
        # per (g, h): flat idx slots + dst_local slots
        for g, bs in enumerate(grp_blocks):
            for h in range(2):
                sl_idx = np.zeros(call_n[g, h], np.int64)
                off = 0
                for b in bs:
                    s_, _ = lists[c][b][h]
                    sl_idx[off:off + len(s_)] = s_
                    off += tiles[b, h] * BLK
                idx_parts.append(_wrap_idx16(sl_idx))
        for t, (g, h, b, slot, _, _) in enumerate(meta):
            s_, d_ = lists[c][b][h]
            boff = sum(tiles[bb, h] for bb in grp_blocks[g] if bb < b) * BLK
            base = slot * BLK - boff  # edge index range within this block+half
            e0, e1 = max(base, 0), min(base + BLK, len(s_))
            if e1 > e0:
                rows = np.arange(e0 - base, e1 - base)
                S[t, rows, d_[e0:e1]] = 1.0
                ST[t, d_[e0:e1], rows] = 1.0
        per_core.append({
            "idxs": np.concatenate(idx_parts, axis=1),
            "S_all": np.ascontiguousarray(S.transpose(1, 0, 2).reshape(BLK, -1)),
            "ST_all": np.ascontiguousarray(ST.transpose(1, 0, 2).reshape(BLK, -1)),
            "deginv": np.ascontiguousarray(
                deg_inv[c].reshape(NB, BLK).T.astype(np.float32)),  # [128, NB]
        })
    idxc = per_core[0]["idxs"].shape[1]
    plan = {"tiles": tiles, "grp_blocks": grp_blocks, "call_n": call_n,
            "meta": meta, "total_tiles": total_tiles, "idxc": idxc,
            "per_core": per_core}
    return plan


def prep_weights(x, W_gat, att_src, att_dst, b_gat, W_l, b_l, W_r):
    """Shared (replicated) input tensors."""
    xp = np.zeros((NPAD, IN), np.float32)
    xp[:N] = x
    W3 = W_gat.reshape(IN, HEADS, HID)
    AsAd = np.concatenate([np.einsum("khj,hj->kh", W3, att_src),
                           np.einsum("khj,hj->kh", W3, att_dst)],
                          axis=1).astype(np.float32)          # [128, 16]
    Wl16 = np.concatenate([W_l[0:128], W_l[128:256]], axis=1)  # [128, 128]
    Wr16 = np.concatenate([W_r[0:128], W_r[128:256]], axis=1)
    return {
        "xT32": np.ascontiguousarray(xp.T),                        # [128, NPAD] f32
        "xT16": np.ascontiguousarray(xp.T.astype(np.float16)),
        "Wg16": W_gat.astype(np.float16),                          # [128, 256]
        "AsAd": AsAd,
        "bgat": np.broadcast_to(b_gat, (BLK, CAT)).astype(np.float32).copy(),
        "Wl16": Wl16.astype(np.float16),
        "Wr16": Wr16.astype(np.float16),
        "bl": np.broadcast_to(b_l, (BLK, OUT)).astype(np.float32).copy(),
        "ident": np.eye(BLK, dtype=np.float16),
    }


# --------------------------------------------------------- bass program -----
def build_program(plan):
    import concourse.bacc as bacc
    import concourse.bass as bass
    import concourse.mybir as mybir
    import concourse.tile as tile
    from concourse import library_config
    from concourse.alu_op_type import AluOpType

    f16, f32, i16 = mybir.dt.float16, mybir.dt.float32, mybir.dt.int16
    EXP = mybir.ActivationFunctionType.Exp

    tiles = plan["tiles"]
    grp_blocks = plan["grp_blocks"]
    call_n = plan["call_n"]
    meta = plan["meta"]
    TT = plan["total_tiles"]
    IDXC = plan["idxc"]

    PH = int(os.environ.get("KPHASES", "9"))
    nc = bacc.Bacc("TRN2", target_bir_lowering=False, debug=False)

    # external inputs
    xT32 = nc.dram_tensor("xT32", [IN, NPAD], f32, kind="ExternalInput")
    xT16 = nc.dram_tensor("xT16", [IN, NPAD], f16, kind="ExternalInput")
    Wg16 = nc.dram_tensor("Wg16", [IN, CAT], f16, kind="ExternalInput")
    AsAd = nc.dram_tensor("AsAd", [IN, 2 * HEADS], f32, kind="ExternalInput")
    bgat = nc.dram_tensor("bgat", [BLK, CAT], f32, kind="ExternalInput")
    Wl16 = nc.dram_tensor("Wl16", [BLK, BLK], f16, kind="ExternalInput")
    Wr16 = nc.dram_tensor("Wr16", [BLK, BLK], f16, kind="ExternalInput")
    bl = nc.dram_tensor("bl", [BLK, OUT], f32, kind="ExternalInput")
    ident = nc.dram_tensor("ident", [BLK, BLK], f16, kind="ExternalInput")
    idxs_d = nc.dram_tensor("idxs", [BLK, IDXC], i16, kind="ExternalInput")
    S_d = nc.dram_tensor("S_all", [BLK, TT * BLK], f16, kind="ExternalInput")
    ST_d = nc.dram_tensor("ST_all", [BLK, TT * BLK], f16, kind="ExternalInput")
    dgi_d = nc.dram_tensor("deginv", [BLK, NB], f32, kind="ExternalInput")

    # external outputs
    x32_o = nc.dram_tensor("x32_out", [PC, CAT], f32, kind="ExternalOutput")
    out2_o = nc.dram_tensor("out2_out", [PC, OUT], f32, kind="ExternalOutput")

    # internal DRAM
    table1 = nc.dram_tensor("table1", [NPAD, TROW], f16)
    adall = nc.dram_tensor("a_d_all", [NPAD, HEADS], f32)
    x32sh = nc.dram_tensor("x32sh", [PC, CAT], f16)
    table2 = nc.dram_tensor("table2", [NPAD, CAT], f16)

    with tile.TileContext(nc) as tc, ExitStack() as ctx:
        nc.gpsimd.load_library(library_config.mlp)
        consts = ctx.enter_context(tc.tile_pool(name="consts", bufs=1))

        def cload(dram, shape, dtype):
            t = consts.tile(shape, dtype, tag=dram.name + "_c")
            nc.sync.dma_start(t[:], dram.ap())
            return t

        wg_t = cload(Wg16, [IN, CAT], f16)
        asad_t = cload(AsAd, [IN, 2 * HEADS], f32)
        bgat_t = cload(bgat, [BLK, CAT], f32)
        wl_t = cload(Wl16, [BLK, BLK], f16)
        wr_t = cload(Wr16, [BLK, BLK], f16)
        bl_t = cload(bl, [BLK, OUT], f32)
        id_t = cload(ident, [BLK, BLK], f16)
        idx_t = cload(idxs_d, [BLK, IDXC], i16)
        dgi_t = cload(dgi_d, [BLK, NB], f32)
        adown = consts.tile([BLK, NB, HEADS], f16, tag="adown")
        x32T = consts.tile([BLK, 2 * NB, BLK], f16, tag="x32T")

        # ---------------- phase 0: h + logits, build table1 ----------------
        with tc.tile_pool(name="p0in", bufs=4) as p0i, \
             tc.tile_pool(name="p0out", bufs=4) as p0o, \
             tc.tile_pool(name="p0ps", bufs=4, space="PSUM") as p0p:
            for k in range(NCHUNK):
                sl = slice(k * BLK, (k + 1) * BLK)
                x16t = p0i.tile([IN, BLK], f16, tag="x16")
                nc.sync.dma_start(x16t[:], xT16.ap()[:, sl])
                x32t = p0i.tile([IN, BLK], f32, tag="x32")
                nc.sync.dma_start(x32t[:], xT32.ap()[:, sl])
                ph = p0p.tile([BLK, CAT], f32, tag="ph")
                nc.tensor.matmul(ph[:], x16t[:], wg_t[:], start=True, stop=True)
                pl = p0p.tile([BLK, 2 * HEADS], f32, tag="pl")
                nc.tensor.matmul(pl[:], x32t[:], asad_t[:], start=True, stop=True)
                trow = p0o.tile([BLK, TROW], f16, tag="trow")
                nc.vector.memset(trow[:, CAT + 2 * HEADS:TROW], 0)
                nc.vector.tensor_copy(trow[:, 0:CAT], ph[:])
                t32 = trow[:].bitcast(f32)
                nc.vector.tensor_copy(t32[:, CAT // 2:CAT // 2 + HEADS],
                                      pl[:, 0:HEADS])
                nc.sync.dma_start(table1.ap()[sl, :], trow[:])
                adt = p0o.tile([BLK, HEADS], f32, tag="adt")
                nc.vector.tensor_copy(adt[:], pl[:, HEADS:2 * HEADS])
                nc.sync.dma_start(adall.ap()[sl, :], adt[:])

        tc.strict_bb_all_engine_barrier()

        # own a_d slice (dynamic offset by core id), cast f32 -> f16 in DMA
        pid = nc.gpsimd.partition_id()
        ad_src = bass.AP(adall, pid * (PC * HEADS),
                         [[HEADS, BLK], [BLK * HEADS, NB], [1, HEADS]])
        nc.gpsimd.dma_start(out=adown[:], in_=ad_src)

        tabA = table1.ap()[0:SPLIT, :]
        tabB = table1.ap()[SPLIT:NPAD, :]

        # ---------------- phase 1: GAT edge pass ----------------
        idx_off = [0]
        for g in range(len(grp_blocks)):
            for h in range(2):
                idx_off.append(idx_off[-1] + int(call_n[g, h]) // 16)
        tile_col = 0  # running mask column (tile index)

        if PH >= 1:
         with tc.tile_pool(name="gath", bufs=2) as gpool, \
             tc.tile_pool(name="mask", bufs=2) as mpool, \
             tc.tile_pool(name="edge", bufs=6) as epool, \
             tc.tile_pool(name="msg", bufs=4) as msgpool, \
             tc.tile_pool(name="blk1", bufs=4) as opool, \
             tc.tile_pool(name="ps1", bufs=4, space="PSUM") as pspool, \
             tc.tile_pool(name="psE", bufs=2, space="PSUM") as pspoolE, \
             tc.tile_pool(name="psT", bufs=2, space="PSUM") as pspoolT:
            t_iter = 0
            K1G = int(os.environ.get("K1G", "9999"))
            for g, bs in enumerate(grp_blocks):
                if g >= K1G:
                    break
                nA, nB_ = int(call_n[g, 0]), int(call_n[g, 1])
                gA = gpool.tile([BLK, nA // BLK, TROW], f16, tag="gA")
                nc.gpsimd.dma_gather(
                    gA[:], tabA, idx_t[:, idx_off[2 * g]:idx_off[2 * g + 1]],
                    nA, nA, TROW, elem_step=TROW, single_packet=False)
                gB = gpool.tile([BLK, nB_ // BLK, TROW], f16, tag="gB")
                nc.gpsimd.dma_gather(
                    gB[:], tabB, idx_t[:, idx_off[2 * g + 1]:idx_off[2 * g + 2]],
                    nB_, nB_, TROW, elem_step=TROW, single_packet=False)
                ntg = sum(int(tiles[b, 0] + tiles[b, 1]) for b in bs)
                sm = mpool.tile([BLK, ntg * BLK], f16, tag="sm")
                nc.sync.dma_start(
                    sm[:], S_d.ap()[:, tile_col * BLK:(tile_col + ntg) * BLK])
                st = mpool.tile([BLK, ntg * BLK], f16, tag="st")
                nc.sync.dma_start(
                    st[:], ST_d.ap()[:, tile_col * BLK:(tile_col + ntg) * BLK])

                K1S = int(os.environ.get("K1S", "9999"))
                psum_b = {}
                for b in bs:
                    psum_b[b] = pspool.tile([BLK, CAT + HEADS], f32, tag="ps1", name="ps1b")
                for tl in range(ntg):
                    g_, h_, b_, slot, first, last = meta[t_iter]
                    assert g_ == g
                    gt = (gA if h_ == 0 else gB)[:, slot, :]
                    msl = slice(tl * BLK, (tl + 1) * BLK)
                    if K1S < 1:
                        t_iter += 1
                        continue
                    # a_d expansion: [128e, 8] = ST.T @ a_d(block)
                    pe_ = pspoolE.tile([BLK, HEADS], f32, tag="pe")
                    nc.tensor.matmul(pe_[:], st[:, msl], adown[:, b_, :],
                                     start=True, stop=True)
                    ev = epool.tile([BLK, HEADS], f32, tag="ev")
                    gt_as = gt.bitcast(f32)[:, CAT // 2:CAT // 2 + HEADS]
                    nc.vector.tensor_add(ev[:], pe_[:], gt_as)
                    if K1S < 2:
                        t_iter += 1
                        continue
                    w0 = epool.tile([BLK, HEADS], f32, tag="w0")
                    nc.vector.scalar_tensor_tensor(
                        w0[:], ev[:], 0.2, ev[:], AluOpType.mult, AluOpType.max)
                    msg = msgpool.tile([BLK, CAT + HEADS], f16, tag="msg")
                    nc.scalar.activation(msg[:, CAT:CAT + HEADS], w0[:], EXP)
                    mo = msg[:, 0:CAT].rearrange("p (h j) -> p h j", h=HEADS)
                    ho = gt[:, 0:CAT].rearrange("p (h j) -> p h j", h=HEADS)
                    wv = msg[:, CAT:CAT + HEADS].unsqueeze(2).broadcast_to(
                        [BLK, HEADS, HID])
                    nc.vector.tensor_mul(mo, ho, wv)
                    if K1S < 3:
                        t_iter += 1
                        continue
                    nc.tensor.matmul(psum_b[b_][:], sm[:, msl], msg[:],
                                     start=first, stop=last)
                    t_iter += 1
                for b in (bs if K1S >= 4 else []):
                    ps = psum_b[b]
                    den = epool.tile([BLK, HEADS], f32, tag="den")
                    nc.vector.tensor_scalar_add(den[:], ps[:, CAT:CAT + HEADS],
                                                1e-16)
                    rec = epool.tile([BLK, HEADS], f32, tag="rec")
                    nc.vector.reciprocal(rec[:], den[:])
                    o32 = opool.tile([BLK, CAT], f32, tag="o32")
                    o3 = o32[:].rearrange("p (h j) -> p h j", h=HEADS)
                    n3 = ps[:, 0:CAT].rearrange("p (h j) -> p h j", h=HEADS)
                    rv = rec[:].unsqueeze(2).broadcast_to([BLK, HEADS, HID])
                    nc.vector.tensor_mul(o3, n3, rv)
                    of = opool.tile([BLK, CAT], f32, tag="of")
                    nc.vector.tensor_add(of[:], o32[:], bgat_t[:])
                    nc.vector.tensor_scalar_max(of[:], of[:], 0.0)
                    o16 = opool.tile([BLK, CAT], f16, tag="o16")
                    nc.vector.tensor_copy(o16[:], of[:])
                    bsl = slice(b * BLK, (b + 1) * BLK)
                    nc.sync.dma_start(x32_o.ap()[bsl, :], of[:])
                    nc.sync.dma_start(x32sh.ap()[bsl, :], o16[:])
                    for half in range(2):
                        pt = pspoolT.tile([BLK, BLK], f32, tag="pt")
                        nc.tensor.matmul(pt[:],
                                         o16[:, half * BLK:(half + 1) * BLK],
                                         id_t[:], start=True, stop=True)
                        nc.vector.tensor_copy(x32T[:, 2 * b + half, :], pt[:])
                tile_col += ntg

        tc.strict_bb_all_engine_barrier()

        # ---------------- allgather x32 (fp16) ----------------
        if PH >= 2:
            nc.gpsimd.collective_compute(
                "AllGather", mybir.AluOpType.bypass,
                replica_groups=[list(range(NCORES))],
                ins=[x32sh.ap().opt()],
                outs=[table2.ap().opt()])

        tc.strict_bb_all_engine_barrier()

        tab2A = table2.ap()[0:SPLIT, :]
        tab2B = table2.ap()[SPLIT:NPAD, :]

        # ---------------- phase 2: SAGE edge pass ----------------
        tile_col = 0
        if PH >= 3:
         with tc.tile_pool(name="gath2", bufs=2) as gpool2, \
             tc.tile_pool(name="mask2", bufs=2) as mpool2, \
             tc.tile_pool(name="blk2", bufs=4) as opool2, \
             tc.tile_pool(name="ps2", bufs=4, space="PSUM") as pspool2, \
             tc.tile_pool(name="psO", bufs=2, space="PSUM") as pspoolO, \
             tc.tile_pool(name="psT2", bufs=2, space="PSUM") as pspoolT2:
            t_iter = 0
            for g, bs in enumerate(grp_blocks):
                nA, nB_ = int(call_n[g, 0]), int(call_n[g, 1])
                gA = gpool2.tile([BLK, nA // BLK, CAT], f16, tag="gA2")
                nc.gpsimd.dma_gather(
                    gA[:], tab2A, idx_t[:, idx_off[2 * g]:idx_off[2 * g + 1]],
                    nA, nA, CAT, elem_step=CAT, single_packet=False)
                gB = gpool2.tile([BLK, nB_ // BLK, CAT], f16, tag="gB2")
                nc.gpsimd.dma_gather(
                    gB[:], tab2B, idx_t[:, idx_off[2 * g + 1]:idx_off[2 * g + 2]],
                    nB_, nB_, CAT, elem_step=CAT, single_packet=False)
                ntg = sum(int(tiles[b, 0] + tiles[b, 1]) for b in bs)
                sm = mpool2.tile([BLK, ntg * BLK], f16, tag="sm2")
                nc.sync.dma_start(
                    sm[:], S_d.ap()[:, tile_col * BLK:(tile_col + ntg) * BLK])

                psum_b = {}
                for b in bs:
                    psum_b[b] = pspool2.tile([BLK, CAT], f32, tag="ps2", name="ps2b")
                for tl in range(ntg):
                    g_, h_, b_, slot, first, last = meta[t_iter]
                    gt = (gA if h_ == 0 else gB)[:, slot, :]
                    nc.tensor.matmul(psum_b[b_][:],
                                     sm[:, tl * BLK:(tl + 1) * BLK], gt,
                                     start=first, stop=last)
                    t_iter += 1
                for b in bs:
                    ps = psum_b[b]
                    m32 = opool2.tile([BLK, CAT], f32, tag="m32")
                    dv = dgi_t[:, b:b + 1].broadcast_to([BLK, CAT])
                    nc.vector.tensor_mul(m32[:], ps[:], dv)
                    m16 = opool2.tile([BLK, CAT], f16, tag="m16")
                    nc.vector.tensor_copy(m16[:], m32[:])
                    pso = pspoolO.tile([BLK, OUT], f32, tag="pso")
                    for half in range(2):
                        pt = pspoolT2.tile([BLK, BLK], f32, tag="pt2")
                        nc.tensor.matmul(pt[:],
                                         m16[:, half * BLK:(half + 1) * BLK],
                                         id_t[:], start=True, stop=True)
                        mt16 = opool2.tile([BLK, BLK], f16, tag="mt16")
                        nc.vector.tensor_copy(mt16[:], pt[:])
                        nc.tensor.matmul(
                            pso[:], mt16[:],
                            wl_t[:, half * OUT:(half + 1) * OUT],
                            start=(half == 0), stop=False)
                    for half in range(2):
                        nc.tensor.matmul(
                            pso[:], x32T[:, 2 * b + half, :],
                            wr_t[:, half * OUT:(half + 1) * OUT],
                            start=False, stop=(half == 1))
                    o2 = opool2.tile([BLK, OUT], f32, tag="o2")
                    nc.vector.tensor_add(o2[:], pso[:], bl_t[:])
                    nc.sync.dma_start(
                        out2_o.ap()[b * BLK:(b + 1) * BLK, :], o2[:])
                tile_col += ntg

    nc.finalize()
    return nc


# ------------------------------------------------------------ entry point ---
_CACHE = {}


def kernel(x, edge_index, W_gat, att_src, att_dst, b_gat, W_l, b_l, W_r):
    from concourse.bass_utils import run_bass_kernel_spmd

    x = np.asarray(x, np.float32)
    edge_index = np.asarray(edge_index)
    plan = prepare(edge_index)
    shared = prep_weights(np.asarray(x, np.float32),
                          np.asarray(W_gat, np.float32),
                          np.asarray(att_src, np.float32),
                          np.asarray(att_dst, np.float32),
                          np.asarray(b_gat, np.float32),
                          np.asarray(W_l, np.float32),
                          np.asarray(b_l, np.float32),
                          np.asarray(W_r, np.float32))

    nc = build_program(plan)

    in_maps = []
    for c in range(NCORES):
        m = dict(shared)
        pc = plan["per_core"][c]
        m["idxs"] = pc["idxs"]
        m["S_all"] = pc["S_all"]
        m["ST_all"] = pc["ST_all"]
        m["deginv"] = pc["deginv"]
        in_maps.append(m)

    res = run_bass_kernel_spmd(nc, in_maps, core_ids=list(range(NCORES)),
                               trace=bool(int(os.environ.get("KTRACE", "0"))))
    kernel._last_results = res

    x32 = np.zeros((N, CAT), np.float32)
    out2 = np.zeros((N, OUT), np.float32)
    for c in range(NCORES):
        lo = c * PC
        hi = min(lo + PC, N)
        if hi > lo:
            x32[lo:hi] = res.results[c]["x32_out"][:hi - lo]
            out2[lo:hi] = res.results[c]["out2_out"][:hi - lo]
    return x32, out2
